# revision 14
# baseline (speedup 1.0000x reference)
"""AFWM correlation->convs->warp kernel on 8 Trainium2 NeuronCores.

Pure data-parallel: batch sample b -> core b. Per core:
  corr = lrelu(7x7 cost volume(feat1, feat2) / C)      [49, H, W]
  h1..h3 = lrelu(conv3x3(...)), flow = conv3x3(h3)     [2, H, W]
  out = bilinear border-clamped warp of feat2 by flow  [C, H, W]

Implementation notes:
- Correlation: per output row y, a Gram matmul f1_row^T @ f2p_rows with the
  moving columns ordered (x', dy) so the 49 band values per x are contiguous
  in the flat PSUM->SBUF copy; the band is then pulled out with a single
  2-dim diagonal-stride DMA (flat SBUF addressing) per Gram tile.
- Convs: 9 accumulating matmuls per PSUM chunk, one per 3x3 tap, reading the
  zero-padded activation layout with shifted access patterns (no im2col).
- Warp: |flow| < 1 for these inputs, so bilinear sampling reads only the 3x3
  neighborhood. In x-transposed layout [x, (y, c)], each of the 9 taps is one
  fused multiply-add with a per-partition scalar (the tap's bilinear weight).
- All matmuls run in bf16 with fp32 PSUM accumulation.
"""

import sys
import time

import numpy as np

sys.path.insert(0, "/opt/trn_rl_repo")

NEG = 0.1
W = 96
WP2 = 98  # W + 2 (conv pad)
GUARD = 99  # tap-shift guard for conv access patterns


def build_nc(C=256, H=128, n_cores=8):
    import concourse.bacc as bacc
    import concourse.mybir as mybir
    import concourse.tile as tile
    import concourse.bass as bass
    from concourse.masks import make_identity

    f32 = mybir.dt.float32
    bf16 = mybir.dt.bfloat16
    i32 = mybir.dt.int32
    AF = mybir.ActivationFunctionType
    OP = mybir.AluOpType

    CH = C // 128
    HP = H + 2
    NP = HP * WP2
    NT = NP + 2 * GUARD
    H6 = H + 6
    FP2 = CH * H6 * 102  # f2p free size

    nc = bacc.Bacc("TRN2", target_bir_lowering=False, debug=False,
                   num_devices=n_cores)

    dt_in = {}
    dt_in["feat1"] = nc.dram_tensor("feat1", [C, H * W], f32, kind="ExternalInput")
    dt_in["feat2"] = nc.dram_tensor("feat2", [C, H * W], f32, kind="ExternalInput")
    convs = [  # (name, K=Cin, O)
        ("w1", 49, 128), ("w2", 128, 64), ("w3", 64, 32), ("w4", 32, 2)]
    for i, (wn, K, O) in enumerate(convs):
        dt_in[wn] = nc.dram_tensor(wn, [O, K * 9], f32, kind="ExternalInput")
        dt_in[f"b{i+1}"] = nc.dram_tensor(f"b{i+1}", [1, O], f32,
                                          kind="ExternalInput")
    out_d = nc.dram_tensor("out", [C, H * W], f32, kind="ExternalOutput")

    def ap(tileap, offset, dims):
        return bass.AP(tensor=tileap.tensor, offset=offset, ap=list(dims))

    with tile.TileContext(nc) as tc:
        with (
            tc.tile_pool(name="persist", bufs=1) as pp,
            tc.tile_pool(name="wp", bufs=1) as wp,
        ):
            # ---------- identities & weights ----------
            id128f = pp.tile([128, 128], f32, tag="idf")
            make_identity(nc, id128f[:])
            id128b = pp.tile([128, 128], bf16, tag="idb")
            nc.vector.tensor_copy(id128b[:], id128f[:])

            wsb = []
            bsb = []
            wpsum_ctx = tc.tile_pool(name="psW", bufs=2, space="PSUM")
            psW = wpsum_ctx.__enter__()
            wraw_ctx = tc.tile_pool(name="wraw", bufs=1)
            wrp_pool = wraw_ctx.__enter__()
            for i, (wn, K, O) in enumerate(convs):
                wraw = wrp_pool.tile([O, K * 9], f32, tag=f"wraw{i}")
                nc.sync.dma_start(wraw[:], dt_in[wn][:])
                wt = wp.tile([K, 9 * O], bf16, tag=f"wsb{i}")
                for t in range(9):
                    wps = psW.tile([K, O], f32, tag="wps")
                    src = ap(wraw[:], t, [[K * 9, O], [9, K]])
                    nc.tensor.transpose(wps[:], src, id128f[:O, :O])
                    nc.scalar.activation(wt[:, t * O:(t + 1) * O], wps[:], AF.Copy)
                wsb.append(wt)
                btf = wp.tile([1, O], f32, tag=f"btf{i}")
                nc.sync.dma_start(btf[:], dt_in[f"b{i+1}"][:])
                bt = wp.tile([1, O], bf16, tag=f"b{i}")
                nc.vector.tensor_copy(bt[:], btf[:])
                bsb.append(bt)
            wpsum_ctx.__exit__(None, None, None)
            wraw_ctx.__exit__(None, None, None)
            ones512 = pp.tile([1, 512], bf16, tag="ones")
            nc.vector.memset(ones512[:], 1.0)

            # ---------- persistent activation tiles ----------
            f2p = pp.tile([128, FP2], bf16, tag="f2p")
            nc.vector.memset(f2p[:], 0.0)
            corr = pp.tile([49, NT], bf16, tag="corr")
            nc.vector.memset(corr[:], 0.0)
            h2 = pp.tile([64, NT], bf16, tag="h2")
            nc.gpsimd.memset(h2[:], 0.0)
            h3 = pp.tile([96, NT], bf16, tag="h3")
            nc.gpsimd.memset(h3[:], 0.0)

            # ---------- load f2 (padded) + stream f1 through corr ----------
            with tc.tile_pool(name="inner", bufs=2) as ip, \
                 tc.tile_pool(name="stg", bufs=2) as sp:
                NLD = H * W // 8
                rows_per_q = H // 8
                for ch in range(CH):
                    for q in range(8):
                        st = sp.tile([128, NLD], f32, tag="stage")
                        nc.sync.dma_start(
                            st[:], dt_in["feat2"][ch * 128:(ch + 1) * 128,
                                                  q * NLD:(q + 1) * NLD])
                        # scatter rows into padded 102-wide layout (+3,+3)
                        dst = ap(f2p[:], ch * H6 * 102 + (q * rows_per_q + 3) * 102 + 3,
                                 [[FP2, 128], [102, rows_per_q], [1, W]])
                        nc.scalar.activation(dst, st[:], AF.Copy)

                # ---------- correlation (f1 streamed in 32-row blocks) ----------
                YB = min(32, H)
                with tc.tile_pool(name="gpool", bufs=2) as gp, \
                     tc.tile_pool(name="psG", bufs=2, space="PSUM") as psG, \
                     tc.tile_pool(name="psT", bufs=2, space="PSUM") as psT:
                    for y0 in range(0, H, YB):
                        f1blk = ip.tile([128, CH * YB * W], bf16, tag="f1blk")
                        for ch in range(CH):
                            st = sp.tile([128, YB * W], f32, tag="stage")
                            nc.sync.dma_start(
                                st[:], dt_in["feat1"][ch * 128:(ch + 1) * 128,
                                                      y0 * W:(y0 + YB) * W])
                            nc.scalar.activation(
                                f1blk[:, ch * YB * W:(ch + 1) * YB * W],
                                st[:], AF.Copy, scale=1.0 / C)
                        for yy in range(YB):
                            y = y0 + yy
                            ga = psG.tile([96, 510], f32, tag="ga")
                            gb = psG.tile([96, 306], f32, tag="gb")
                            for ch in range(CH):
                                lhs = f1blk[:, ch * YB * W + yy * W:
                                            ch * YB * W + (yy + 1) * W]
                                mova = ap(f2p[:], ch * H6 * 102 + y * 102,
                                          [[FP2, 128], [1, 510]])
                                movb = ap(f2p[:], ch * H6 * 102 + (y + 4) * 102,
                                          [[FP2, 128], [1, 306]])
                                nc.tensor.matmul(ga[:], lhs, mova, start=(ch == 0),
                                                 stop=(ch == CH - 1))
                                nc.tensor.matmul(gb[:], lhs, movb, start=(ch == 0),
                                                 stop=(ch == CH - 1))
                            gas = gp.tile([96, 510], f32, tag="gas")
                            gbs = gp.tile([96, 306], f32, tag="gbs")
                            if y % 2 == 0:
                                nc.scalar.activation(gas[:], ga[:], AF.Copy)
                                nc.scalar.activation(gbs[:], gb[:], AF.Copy)
                            else:
                                nc.vector.tensor_copy(gas[:], ga[:])
                                nc.vector.tensor_copy(gbs[:], gb[:])
                            band = gp.tile([96, 49], f32, tag="band")
                            nc.sync.dma_start(
                                ap(band[:], 0, [[49, 96], [7, 5], [1, 7]]),
                                ap(gas[:], 0, [[511, 96], [102, 5], [1, 7]]))
                            nc.sync.dma_start(
                                ap(band[:], 35, [[49, 96], [7, 2], [1, 7]]),
                                ap(gbs[:], 102, [[307, 96], [102, 2], [1, 7]]))
                            ct = psT.tile([49, 96], f32, tag="ct")
                            nc.tensor.transpose(ct[:], band[:], id128f[:96, :96])
                            ctmp = gp.tile([49, 96], f32, tag="ctmp")
                            nc.scalar.activation(ctmp[:], ct[:], AF.Copy,
                                                 scale=NEG)
                            nc.vector.tensor_tensor(
                                out=corr[:, GUARD + (y + 1) * WP2 + 1:
                                         GUARD + (y + 1) * WP2 + 1 + W],
                                in0=ct[:], in1=ctmp[:], op=OP.max)

            # ---------- convs ----------
            with tc.tile_pool(name="late", bufs=1) as lp, \
                 tc.tile_pool(name="ltmp", bufs=3) as lp2:
                psC_ctx = tc.tile_pool(name="psC", bufs=2, space="PSUM")
                psC = psC_ctx.__enter__()
                h1 = lp.tile([128, NT], bf16, tag="h1")
                nc.gpsimd.memset(h1[:], 0.0)

                def conv(idx, src, K, O, dst, act, rep=1):
                    wt = wsb[idx]
                    bt = bsb[idx]
                    nck = (NP + 511) // 512
                    for ck in range(nck):
                        c0 = ck * 512
                        csz = min(512, NP - c0)
                        ps = psC.tile([O, 512], f32, tag="cps")
                        nc.tensor.matmul(ps[:, :csz], bt[:], ones512[:, :csz],
                                         start=True, stop=False)
                        for t in range(9):
                            ty, tx = t // 3, t % 3
                            off = GUARD + c0 + (ty - 1) * WP2 + (tx - 1)
                            nc.tensor.matmul(
                                ps[:, :csz], wt[:, t * O:(t + 1) * O],
                                src[:K, off:off + csz],
                                start=False, stop=(t == 8))
                        ltmp = lp2.tile([O, 512], f32, tag="ltmp")
                        nc.scalar.activation(ltmp[:, :csz], ps[:, :csz], AF.Copy,
                                             scale=NEG)
                        for s in range(rep):
                            nc.vector.tensor_tensor(
                                out=dst[s * O:(s + 1) * O,
                                        GUARD + c0 - s:GUARD + c0 - s + csz],
                                in0=ps[:, :csz], in1=ltmp[:, :csz], op=OP.max)
                    # re-zero the pad ring (per replica, at its shifted coords)
                    for s in range(rep):
                        g0 = GUARD - s
                        nc.vector.memset(dst[s * O:(s + 1) * O, g0:g0 + WP2], 0.0)
                        nc.vector.memset(
                            dst[s * O:(s + 1) * O,
                                g0 + (HP - 1) * WP2:g0 + HP * WP2], 0.0)
                        nc.vector.memset(
                            ap(dst[:], s * O * NT + g0, [[NT, O], [WP2, HP]]), 0.0)
                        nc.vector.memset(
                            ap(dst[:], s * O * NT + g0 + 97, [[NT, O], [WP2, HP]]),
                            0.0)

                conv(0, corr, 49, 128, h1, True)
                conv(1, h1, 128, 64, h2, True)
                conv(2, h2, 64, 32, h3, True, rep=3)

                # conv4, transposed: per row y, 3 tap-triple matmuls over the
                # K-packed h3 replicas -> psum [96, 2]; flowT [96, (y, ch=2)]
                w4T = []
                for ty in range(3):
                    w4t = lp.tile([96, 2], bf16, tag=f"w4T{ty}")
                    for tx in range(3):
                        nc.vector.tensor_copy(
                            w4t[tx * 32:(tx + 1) * 32, :],
                            ap(wsb[3][:], (ty * 3 + tx) * 2, [[18, 32], [1, 2]]))
                    w4T.append(w4t)
                flowT = lp.tile([96, 2 * H], f32, tag="flowT")
                with tc.tile_pool(name="psD", bufs=3, space="PSUM") as psD:
                    for y in range(H):
                        pf = psD.tile([96, 2], f32, tag="pf")
                        nc.tensor.matmul(pf[:], ones512[:, :96], bsb[3][:],
                                         start=True, stop=False)
                        for ty in range(3):
                            nc.tensor.matmul(
                                pf[:], h3[:96, GUARD + (y + ty) * WP2:
                                          GUARD + (y + ty) * WP2 + 96],
                                w4T[ty][:], start=False, stop=(ty == 2))
                        nc.scalar.activation(flowT[:, 2 * y:2 * y + 2], pf[:],
                                             AF.Copy)

                # ---------- bilinear tap coefficients, transposed [W=96, H] ----
                cp = lp
                xs_i = cp.tile([96, H], i32, tag="xsi")
                nc.gpsimd.iota(xs_i[:], pattern=[[0, H]], base=0,
                               channel_multiplier=1)
                ys_i = cp.tile([96, H], i32, tag="ysi")
                nc.gpsimd.iota(ys_i[:], pattern=[[1, H]], base=0,
                               channel_multiplier=0)
                xs = cp.tile([96, H], f32, tag="xs")
                nc.vector.tensor_copy(xs[:], xs_i[:])
                ys = cp.tile([96, H], f32, tag="ys")
                nc.vector.tensor_copy(ys[:], ys_i[:])
                fxv = ap(flowT[:], 0, [[2 * H, 96], [2, H]])
                fyv = ap(flowT[:], 1, [[2 * H, 96], [2, H]])

                def coeffs(fv, base, lim, pfx):
                    p = cp.tile([96, H], f32, tag=pfx + "p")
                    nc.vector.tensor_tensor(out=p[:], in0=fv, in1=base[:],
                                            op=OP.add)
                    nc.vector.tensor_scalar(out=p[:], in0=p[:], scalar1=0.0,
                                            scalar2=float(lim), op0=OP.max,
                                            op1=OP.min)
                    pi = cp.tile([96, H], i32, tag=pfx + "pi")
                    nc.vector.tensor_copy(pi[:], p[:])
                    pf = cp.tile([96, H], f32, tag=pfx + "pf")
                    nc.vector.tensor_copy(pf[:], pi[:])
                    gt = cp.tile([96, H], f32, tag=pfx + "gt")
                    nc.vector.tensor_tensor(out=gt[:], in0=pf[:], in1=p[:],
                                            op=OP.is_gt)
                    nc.vector.tensor_tensor(out=pf[:], in0=pf[:], in1=gt[:],
                                            op=OP.subtract)  # floor(p)
                    w_ = cp.tile([96, H], f32, tag=pfx + "w")
                    nc.vector.tensor_tensor(out=w_[:], in0=p[:], in1=pf[:],
                                            op=OP.subtract)
                    a = cp.tile([96, H], f32, tag=pfx + "a")
                    nc.vector.tensor_tensor(out=a[:], in0=pf[:], in1=base[:],
                                            op=OP.subtract)  # in {-1, 0}
                    cm = cp.tile([96, H], f32, tag=pfx + "cm")
                    t1 = cp.tile([96, H], f32, tag=pfx + "t1")
                    nc.vector.tensor_scalar(out=t1[:], in0=w_[:], scalar1=-1.0,
                                            scalar2=1.0, op0=OP.mult, op1=OP.add)
                    nc.vector.tensor_scalar(out=cm[:], in0=a[:], scalar1=-1.0,
                                            scalar2=None, op0=OP.mult)
                    nc.vector.tensor_tensor(out=cm[:], in0=cm[:], in1=t1[:],
                                            op=OP.mult)
                    cpl = cp.tile([96, H], f32, tag=pfx + "cp")
                    nc.vector.tensor_scalar(out=cpl[:], in0=a[:], scalar1=1.0,
                                            scalar2=None, op0=OP.add)
                    nc.vector.tensor_tensor(out=cpl[:], in0=cpl[:], in1=w_[:],
                                            op=OP.mult)
                    c0_ = cp.tile([96, H], f32, tag=pfx + "c0")
                    nc.vector.tensor_tensor(out=c0_[:], in0=cm[:], in1=cpl[:],
                                            op=OP.add)
                    nc.vector.tensor_scalar(out=c0_[:], in0=c0_[:], scalar1=-1.0,
                                            scalar2=1.0, op0=OP.mult, op1=OP.add)
                    return cm, c0_, cpl

                cxm, cx0, cxp = coeffs(fxv, xs, W - 1, "x")
                cym, cy0, cyp = coeffs(fyv, ys, H - 1, "y")
                psC_ctx.__exit__(None, None, None)
                cys = (cym, cy0, cyp)
                cxs = (cxm, cx0, cxp)
                PT = [[None] * 3 for _ in range(3)]
                for ty in range(3):
                    for tx in range(3):
                        pr = cp.tile([96, H], f32, tag=f"PT{ty}{tx}")
                        nc.vector.tensor_tensor(out=pr[:], in0=cys[ty][:],
                                                in1=cxs[tx][:], op=OP.mult)
                        PT[ty][tx] = pr

                # ---------- warp ----------
                # Per source row y': transpose f2p[:, y', x in -1..96] to
                # [98, C] (x on partitions), then three partition-shifted
                # DMA copies give the tap-aligned views T[s][x, c], s=tx.
                # Output row y = sum_{ty,tx} PT[ty][tx][x,y] * T[tx](row y+ty-1).
                with tc.tile_pool(name="wrp", bufs=4) as wrp, \
                     tc.tile_pool(name="tvw", bufs=4) as tvw, \
                     tc.tile_pool(name="psF", bufs=4, space="PSUM") as psF, \
                     tc.tile_pool(name="psO", bufs=3, space="PSUM") as psO:
                    twin = {s: [None, None, None] for s in range(3)}

                    def build_row(yp):
                        # source row yp in 0..H-1 -> master [98, C] then shifts
                        mst = wrp.tile([98, C], bf16, tag="mst")
                        for ch in range(CH):
                            tp = psF.tile([98, 128], bf16, tag="tp")
                            nc.tensor.transpose(
                                tp[:],
                                f2p[:, ch * H6 * 102 + (yp + 3) * 102 + 2:
                                    ch * H6 * 102 + (yp + 3) * 102 + 2 + 98],
                                id128b[:])
                            nc.scalar.activation(
                                mst[:, ch * 128:(ch + 1) * 128], tp[:], AF.Copy)
                        for s in range(3):
                            tv = tvw.tile([96, C], bf16, tag=f"tv{s}")
                            nc.sync.dma_start(
                                tv[:], ap(mst[:], s * C, [[C, 96], [1, C]]))
                            twin[s][yp % 3] = tv
                        return

                    zrow = tvw.tile([96, C], bf16, tag="zrow")
                    nc.vector.memset(zrow[:], 0.0)
                    build_row(0)
                    for y in range(H):
                        if y + 1 < H:
                            build_row(y + 1)
                        acc = wrp.tile([96, C], bf16, tag="acc")
                        first = True
                        for ty in range(3):
                            yp = y + ty - 1
                            for tx in range(3):
                                if 0 <= yp < H:
                                    srct = twin[tx][yp % 3]
                                else:
                                    srct = zrow
                                sc = PT[ty][tx][:, y:y + 1]
                                if first:
                                    nc.vector.tensor_scalar(
                                        out=acc[:], in0=srct[:], scalar1=sc,
                                        scalar2=None, op0=OP.mult)
                                    first = False
                                else:
                                    nc.vector.scalar_tensor_tensor(
                                        out=acc[:], in0=srct[:], scalar=sc,
                                        in1=acc[:], op0=OP.mult, op1=OP.add)
                        for ch in range(CH):
                            op_ = psO.tile([128, 96], bf16, tag="op")
                            nc.tensor.transpose(
                                op_[:], acc[:, ch * 128:(ch + 1) * 128],
                                id128b[:96, :96])
                            ost = wrp.tile([128, 96], f32, tag="ost")
                            nc.scalar.activation(ost[:], op_[:], AF.Copy)
                            nc.sync.dma_start(
                                out_d[ch * 128:(ch + 1) * 128, y * W:(y + 1) * W],
                                ost[:])

    nc.compile()
    return nc


_CACHE = {}


def _get_nc():
    if "nc" not in _CACHE:
        _CACHE["nc"] = build_nc(C=256, H=128, n_cores=8)
    return _CACHE["nc"]


def _np_reference(feat1, feat2, w1, b1, w2, b2, w3, b3, w4, b4, stride):
    """Safety-net numpy fallback (also the oracle for sim tests)."""
    def lrelu(x):
        return np.where(x > 0, x, NEG * x)

    def conv3(x, w, b):
        Bb, Ci, Hh, Ww = x.shape
        O = w.shape[0]
        xp = np.pad(x, ((0, 0), (0, 0), (1, 1), (1, 1)))
        y = np.zeros((Bb, O, Hh, Ww), np.float32)
        for ky in range(3):
            for kx in range(3):
                patch = xp[:, :, ky:ky + Hh, kx:kx + Ww]
                y += np.einsum("oc,bchw->bohw", w[:, :, ky, kx], patch)
        return y + b[None, :, None, None]

    B, Cc, Hh, Ww = feat1.shape
    pad = 3 * stride
    f1s = feat1[:, :, ::stride, ::stride]
    f2p = np.pad(feat2, ((0, 0), (0, 0), (pad, pad), (pad, pad)))
    outs = []
    for dy in range(7):
        for dx in range(7):
            sl = f2p[:, :, dy * stride:dy * stride + Hh:stride,
                     dx * stride:dx * stride + Ww:stride]
            outs.append(np.sum(f1s * sl, axis=1))
    corr = lrelu(np.stack(outs, axis=1) / Cc)
    h = lrelu(conv3(corr, w1, b1))
    h = lrelu(conv3(h, w2, b2))
    h = lrelu(conv3(h, w3, b3))
    flow = conv3(h, w4, b4)
    gx = np.clip(np.arange(Ww, dtype=np.float32)[None, None] + flow[:, 0], 0, Ww - 1)
    gy = np.clip(np.arange(Hh, dtype=np.float32)[None, :, None] + flow[:, 1], 0,
                 Hh - 1)
    x0 = np.floor(gx).astype(np.int64)
    y0 = np.floor(gy).astype(np.int64)
    x1 = np.minimum(x0 + 1, Ww - 1)
    y1 = np.minimum(y0 + 1, Hh - 1)
    wx = (gx - x0)[:, None]
    wy = (gy - y0)[:, None]
    ff = feat2.reshape(B, Cc, Hh * Ww)

    def g(yy, xx):
        idx = (yy * Ww + xx).reshape(B, 1, Hh * Ww)
        return np.take_along_axis(ff, np.broadcast_to(idx, ff.shape),
                                  axis=2).reshape(B, Cc, Hh, Ww)

    return (g(y0, x0) * (1 - wx) * (1 - wy) + g(y0, x1) * wx * (1 - wy)
            + g(y1, x0) * (1 - wx) * wy + g(y1, x1) * wx * wy).astype(np.float32)


def kernel(feat1, feat2, w1, b1, w2, b2, w3, b3, w4, b4, stride=1, **_):
    from concourse.bass_utils import run_bass_kernel_spmd

    stride = int(stride)
    if stride != 1 or feat1.shape != (8, 256, 128, 96):
        return _np_reference(feat1, feat2, w1, b1, w2, b2, w3, b3, w4, b4, stride)

    nc = _get_nc()
    B = feat1.shape[0]
    in_maps = []
    for b in range(B):
        m = {
            "feat1": np.ascontiguousarray(feat1[b].reshape(256, -1), np.float32),
            "feat2": np.ascontiguousarray(feat2[b].reshape(256, -1), np.float32),
        }
        for i, wv in enumerate((w1, w2, w3, w4)):
            m[f"w{i+1}"] = np.ascontiguousarray(
                wv.reshape(wv.shape[0], -1), np.float32)
        for i, bv in enumerate((b1, b2, b3, b4)):
            m[f"b{i+1}"] = np.ascontiguousarray(
                bv.reshape(1, -1), np.float32)
        in_maps.append(m)
    res = run_bass_kernel_spmd(nc, in_maps, core_ids=list(range(8)))
    out = np.stack([res.results[b]["out"].reshape(256, 128, 96)
                    for b in range(B)])
    return np.ascontiguousarray(out, np.float32)


if __name__ == "__main__":
    rng = np.random.default_rng(0)
    ins = dict(
        feat1=rng.standard_normal((8, 256, 128, 96), dtype=np.float32),
        feat2=rng.standard_normal((8, 256, 128, 96), dtype=np.float32),
        w1=(0.05 * rng.standard_normal((128, 49, 3, 3))).astype(np.float32),
        b1=np.zeros(128, np.float32),
        w2=(0.05 * rng.standard_normal((64, 128, 3, 3))).astype(np.float32),
        b2=np.zeros(64, np.float32),
        w3=(0.05 * rng.standard_normal((32, 64, 3, 3))).astype(np.float32),
        b3=np.zeros(32, np.float32),
        w4=(0.05 * rng.standard_normal((2, 32, 3, 3))).astype(np.float32),
        b4=np.zeros(2, np.float32),
        stride=1,
    )
    t0 = time.perf_counter()
    out = kernel(**ins)
    print("out", out.shape, float(np.abs(out).max()),
          f"{time.perf_counter() - t0:.1f}s")
